# revision 20
# baseline (speedup 1.0000x reference)
"""Trainium2 Bass kernel for nn_MultiHeadAttention_64106681860559.

Fused single-score-matrix MHA: qkv = x@Wqkv+b; S = q k^T/8; attn = softmax(S);
out = (attn @ v) @ Wout + bout.   x:[4096,1024] fp32 -> y:[4096,1024] fp32.

Strategy: shard queries (dim 0) across 8 cores; ZERO collectives via weight
folding (associativity):
  scores^T = K Q^T = x (Wk Wq^T) x_own^T   with G = Wk Wq^T folded on host,
  so per core: P = G^T-chunks @ x_own^T  [1024, 512], then S^T = x @ P using
  the full (replicated) x — no K/V AllGather needed.  The key-side bias
  (x_j . Wk bq) folds into P's bias add; query-side constants cancel in
  softmax.  Attention output:
  y^T = (Wv Wo)^T (x^T E) * (1/d) + (bv Wo + bo)  with W2 = Wv Wo folded on
  host; T = x^T E is accumulated unnormalized and the per-query 1/d scale is
  folded into T's final fp16 conversion (T*rb is the normalized attention
  output, O(1), fp16-safe), so the y evacuation is a single bias-add.
Token rotation: each core's x/x^T host layouts are rolled by i*512 tokens so
its OWN tokens are always keys 0..511 — softmax and attn@v are key-order
invariant, so this is exact, and it removes the separate x_own load while
letting the first P matmuls start after just cp|gt_a0|xt-half0 (~0.77 MB).
Schedule: PE warmup spins during the fixed kernel-entry + first-DMA window
(keeps the HAM clock gate open), critical loads are split across BOTH HW DGE
queues (sync + scalar) in exact consumption order, all DMA sources are
host-relaid-out for >=2KB contiguous runs.
Per-core PE work: P (32768 cyc) + S^T (131072) + T (131072) + y^T (32768)
= 327680 cycles of fp16 matmul (~137 us at 2.4 GHz).
"""
import sys
import numpy as np

for _p in ("/opt/trn_rl_repo", "/root/.axon_site/_ro/trn_rl_repo"):
    if _p not in sys.path:
        sys.path.insert(0, _p)

import concourse.bass as bass  # noqa: E402
import concourse.tile as tile  # noqa: E402
from concourse import bacc, mybir  # noqa: E402
from concourse.bass_utils import run_bass_kernel_spmd  # noqa: E402

R = 8            # cores
N = 4096         # tokens
S = N // R       # 512 queries per shard
SH = S // 2      # 256-query half
L = 1024         # latent
KO = L // 128    # 8 latent chunks
NKC = N // 128   # 32 key chunks
NQT = 4          # key-chunk quarters (8 chunks each)
NB = N // 512    # 8 token blocks of 512
EXP_SHIFT = -16.0
SCALE = 0.125    # 1/sqrt(Dk)
WARMUP = 100

f16 = mybir.dt.float16
f32 = mybir.dt.float32

_cached = None


def _build():
    nc = bacc.Bacc("TRN2", target_bir_lowering=False, debug=False, num_devices=R)

    # all host views pre-laid-out partition-major for contiguous DMA
    cp = nc.dram_tensor("cp", [128, KO], f16, kind="ExternalInput")
    gta0 = nc.dram_tensor("gta0", [128, KO * 128], f16, kind="ExternalInput")
    gtr = nc.dram_tensor("gtr", [128, 7 * KO * 128], f16, kind="ExternalInput")
    # xt0: own-token block, piece-major: [p][h][lo][256]
    xt0 = nc.dram_tensor("xt0", [128, 2 * KO * SH], f16, kind="ExternalInput")
    # xtr: blocks 1..7, [p][b][lo][512]
    xtr = nc.dram_tensor("xtr", [128, 7 * KO * 512], f16, kind="ExternalInput")
    xtok = nc.dram_tensor("xtok", [128, NKC * L], f16, kind="ExternalInput")
    w2t = nc.dram_tensor("w2t", [128, KO * L], f16, kind="ExternalInput")
    b2 = nc.dram_tensor("b2", [128, KO], f32, kind="ExternalInput")
    yT = nc.dram_tensor("yT", [L, S], f32, kind="ExternalOutput")

    with tile.TileContext(nc) as tc:
        with tc.tile_pool(name="const", bufs=1) as const, \
             tc.tile_pool(name="xkpool", bufs=2) as xkpool, \
             tc.tile_pool(name="epool", bufs=16) as epool, \
             tc.tile_pool(name="ypool", bufs=3) as ypool, \
             tc.tile_pool(name="ps_a", bufs=2, space="PSUM") as ps_a, \
             tc.tile_pool(name="ps_s", bufs=3, space="PSUM") as ps_s_pool, \
             tc.tile_pool(name="ps_t", bufs=3, space="PSUM") as ps_t_pool:

            # ---- PE warmup: dummy matmuls during the fixed kernel-entry +
            #      first-DMA window keep the HAM activity monitor busy so P
            #      runs at full clock from its first instruction ----
            warm16 = const.tile([128, 64], f16, name="warm16")
            nc.vector.memset(warm16[:], 0.0)
            ps_w0 = ps_a.tile([128, 64], f32, tag="ps_a", name="ps_w0")
            ps_w1 = ps_a.tile([128, 64], f32, tag="ps_a", name="ps_w1")
            for i in range(WARMUP):
                ps_w = ps_w0 if (i & 1) == 0 else ps_w1
                nc.tensor.matmul(ps_w[:64, :], warm16[:, :64],
                                 warm16[:, :64], start=True, stop=True)

            # ---- sync-queue (q1) loads, strict need order.  Early DMA
            #      bandwidth is a shared ~220 GB/s pool no matter how many
            #      queues are active, so ALL head-critical bytes go on ONE
            #      queue in exact consumption order; the scalar queue's bulk
            #      is held back until S^T starts (see below) ----
            cp16v = const.tile([128, KO], f16, name="cp16v")
            nc.sync.dma_start(cp16v[:], cp.ap())
            gta0_t = const.tile([128, KO, 128], f16, name="gta0_t")
            gta0_view = gta0.ap().rearrange("p (bo la) -> p bo la", la=128)
            # own-token block: [p][h][lo][256]
            xt0_t = const.tile([128, 2, KO, SH], f16, name="xt0_t")
            xt0_view = xt0.ap().rearrange(
                "p (h lo t) -> p h lo t", h=2, t=SH)
            nc.sync.dma_start(gta0_t[:], gta0_view)
            nc.sync.dma_start(xt0_t[:, 0], xt0_view[:, 0])
            gt16 = const.tile([128, 7, KO, 128], f16, name="gt16")
            gtr_view = gtr.ap().rearrange(
                "p (a bo la) -> p a bo la", a=7, la=128)
            for a in range(7):          # a = 1..7, all on sync queue
                nc.sync.dma_start(gt16[:, a], gtr_view[:, a])
            nc.sync.dma_start(xt0_t[:, 1], xt0_view[:, 1])

            cp_s = const.tile([128, KO], f32, name="cp_s")
            nc.vector.tensor_copy(out=cp_s[:], in_=cp16v[:])

            ones_c16 = const.tile([128, 1], f16, name="ones_c16")
            nc.vector.memset(ones_c16[:], 1.0)
            ones_r16 = const.tile([1, 128], f16, name="ones_r16")
            nc.vector.memset(ones_r16[:], 1.0)
            expb = const.tile([128, 1], f32, name="expb")
            nc.vector.memset(expb[:], EXP_SHIFT)

            P16 = const.tile([128, KO, S], f16, name="P16")
            xt16 = const.tile([128, 7, KO, 512], f16, name="xt16")
            t_sum = const.tile([128, S], f32, name="t_sum")
            T_sb = const.tile([128, KO, S], f32, name="T_sb")
            T16 = const.tile([128, KO, S], f16, name="T16")
            rb32 = const.tile([128, S], f32, name="rb32")
            recip16 = const.tile([1, S], f16, name="recip16")

            xtr_view = xtr.ap().rearrange(
                "p (b lo t) -> p b lo t", b=7, t=512)

            def load_xt(b):  # blocks 1..7
                nc.sync.dma_start(xt16[:, b - 1], xtr_view[:, b - 1])

            for b in range(1, 8):
                load_xt(b)
            b2_s = const.tile([128, KO], f32, name="b2_s")
            nc.sync.dma_start(b2_s[:], b2.ap())

            # ---- scalar-queue (q10) bulk: triggers are EMITTED later (after
            #      the first S^T exp on the scalar engine) so they can't
            #      steal early DMA bandwidth from the critical path ----
            xtok_view = xtok.ap().rearrange("p (kc l) -> p kc l", l=L)
            xq_tiles = []

            def load_xq(qt, gate=None):
                xq = xkpool.tile([128, 8, L], f16, tag="xq", name=f"xq{qt}")
                if gate is not None:
                    # 1-elem marker copy: forces the Tile scheduler to hold
                    # this trigger until `gate` is written (exp of an early
                    # S^T chain), keeping the early DMA pool for the
                    # head-critical sync-queue loads
                    nc.vector.tensor_copy(out=xq[0:1, 0, 0:1],
                                          in_=gate[0:1, 0:1])
                nc.scalar.dma_start(xq[:], xtok_view[:, qt * 8:(qt + 1) * 8, :])
                xq_tiles.append(xq)

            w2t16 = const.tile([128, KO, L], f16, name="w2t16")

            # ---- phase P (query-half h): P[:,:,h] = G x_own^T + cvec ----
            # own tokens are xt block 0 (host-rotated), piece h = queries half
            def p_chunk(a, h):
                ps = ps_a.tile([128, SH], f32, tag="ps_a", name="ps_p")
                for bo in range(KO):
                    lhs = (gta0_t[:, bo, :] if a == 0
                           else gt16[:, a - 1, bo, :])
                    nc.tensor.matmul(
                        ps[:], lhs, xt0_t[:, h, bo, :],
                        start=(bo == 0), stop=(bo == KO - 1))
                nc.vector.tensor_scalar_add(
                    out=P16[:, a, h * SH:(h + 1) * SH], in0=ps[:],
                    scalar1=cp_s[:, a:a + 1])

            # xt slice for key-chunk kc: kc 0..3 live in xt0_t, rest in xt16
            def xt_slice(kc, lo):
                if kc < 2:
                    return xt0_t[:, 0, lo, kc * 128:(kc + 1) * 128]
                if kc < 4:
                    return xt0_t[:, 1, lo, (kc - 2) * 128:(kc - 1) * 128]
                b = kc // 4
                c = kc % 4
                return xt16[:, b - 1, lo, c * 128:(c + 1) * 128]

            # ---- S^T chunk (full or half width) + exp + t_sum ----
            tsum_started = [False, False]
            e_tiles = {}

            def st_chain(kc, h=None):
                c0, c1 = (0, S) if h is None else (h * SH, (h + 1) * SH)
                w = c1 - c0
                ps_s = ps_s_pool.tile([128, S], f32, tag="ps_s", name="ps_s")
                for lo in range(KO):
                    nc.tensor.matmul(
                        ps_s[:, :w], xt_slice(kc, lo),
                        P16[:, lo, c0:c1],
                        start=(lo == 0), stop=(lo == KO - 1))
                if kc in e_tiles:
                    e_t = e_tiles[kc]
                else:
                    e_t = epool.tile([128, S], f16, tag="e", name="e_t")
                    e_tiles[kc] = e_t
                nc.scalar.activation(
                    e_t[:, c0:c1], ps_s[:, :w],
                    mybir.ActivationFunctionType.Exp,
                    bias=expb[:], scale=SCALE)
                if h is None and all(tsum_started):
                    nc.vector.tensor_tensor(
                        t_sum[:], t_sum[:], e_t[:], mybir.AluOpType.add)
                else:
                    for hh in ([0, 1] if h is None else [h]):
                        hc0, hc1 = hh * SH, (hh + 1) * SH
                        if not tsum_started[hh]:
                            nc.vector.tensor_copy(
                                out=t_sum[:, hc0:hc1], in_=e_t[:, hc0:hc1])
                            tsum_started[hh] = True
                        else:
                            nc.vector.tensor_tensor(
                                t_sum[:, hc0:hc1], t_sum[:, hc0:hc1],
                                e_t[:, hc0:hc1], mybir.AluOpType.add)

            def t_pass(qt, xq, lo_range=None):
                last = (qt == NQT - 1)
                for lo in (lo_range if lo_range is not None else range(KO)):
                    ps_t = ps_t_pool.tile([128, S], f32, tag="ps_t", name="ps_t")
                    for j in range(8):
                        kc = qt * 8 + j
                        nc.tensor.matmul(
                            ps_t[:], xq[:, j, lo * 128:(lo + 1) * 128],
                            e_tiles[kc][:],
                            start=(j == 0), stop=(j == 7))
                    if qt == 0:
                        nc.vector.tensor_copy(out=T_sb[:, lo, :], in_=ps_t[:])
                    elif not last:
                        nc.vector.tensor_tensor(
                            T_sb[:, lo, :], T_sb[:, lo, :], ps_t[:],
                            mybir.AluOpType.add)
                    else:   # final quarter: add, then fold 1/d into the
                            # fp16 convert (T*rb = normalized attn output)
                        nc.vector.tensor_tensor(
                            T_sb[:, lo, :], T_sb[:, lo, :], ps_t[:],
                            mybir.AluOpType.add)
                        nc.vector.tensor_tensor(
                            T16[:, lo, :], T_sb[:, lo, :], rb32[:],
                            mybir.AluOpType.mult)

            # ---- pipeline: P h0 -> S^T(q0,h0) -> P h1 -> S^T(q0,h1) ->
            #      T(q0) -> S^T(q1) -> T(q1) -> S^T(q2) -> S^T(q3) ->
            #      T(q2) -> T(q3).  The last 16 exps get the whole T(q2/q3)
            #      window to drain before the denominator reads t_sum ----
            for a in range(KO):
                p_chunk(a, 0)
            for kc in range(5):
                st_chain(kc, h=0)
            # bulk loads, each gated behind an early exp via marker copies
            for qt in range(NQT):
                load_xq(qt, gate=e_tiles[qt])
            nc.vector.tensor_copy(out=w2t16[0:1, 0, 0:1],
                                  in_=e_tiles[4][0:1, 0:1])
            nc.scalar.dma_start(
                w2t16[:], w2t.ap().rearrange("p (fo m) -> p fo m", m=L))
            for kc in range(5, 8):
                st_chain(kc, h=0)
            for a in range(KO):
                p_chunk(a, 1)
            for kc in range(8):
                st_chain(kc, h=1)
            t_pass(0, xq_tiles[0])
            for kc in range(8, 16):
                st_chain(kc)
            t_pass(1, xq_tiles[1])
            for kc in range(16, 32):
                st_chain(kc)

            # ---- denominator chain, interleaved so the slow one-lane
            #      reciprocal (~3.3us, on idle ps_a banks to avoid PSUM
            #      port contention) hides under t_pass(2), and the
            #      broadcast matmul issues only after it completes ----
            t_pass(2, xq_tiles[2], lo_range=range(0, 1))
            # fp16 copy of t_sum so the denominator sum-matmul takes the
            # fast fp16 path (fp32 stationary forces the 4x-slower mode)
            t_sum16 = const.tile([128, S], f16, name="t_sum16")
            with nc.allow_low_precision(reason="softmax denominator sum in fp16; 2^-11 rel err far under tolerance"):
                nc.vector.tensor_copy(out=t_sum16[:], in_=t_sum[:])
            psum_d = ps_a.tile([1, S], f32, tag="ps_a", name="psum_d")
            nc.tensor.matmul(psum_d[:], ones_c16[:], t_sum16[:],
                             start=True, stop=True)
            with nc.allow_low_precision(reason="1/d broadcast in fp16; 2^-11 rel err on softmax denominators is far under tolerance"):
                nc.vector.reciprocal(out=recip16[:], in_=psum_d[:])
            t_pass(2, xq_tiles[2], lo_range=range(1, KO))
            ps_bc = ps_a.tile([128, S], f32, tag="ps_a", name="ps_bc")
            nc.tensor.matmul(ps_bc[:], ones_r16[:], recip16[:],
                             start=True, stop=True)
            nc.vector.tensor_copy(out=rb32[:], in_=ps_bc[:])
            t_pass(3, xq_tiles[3])

            # ---- output projection: yT = W2^T (T*rb) + b2; the 1/d scale
            #      was folded into T16, so evacuation is one bias-add ----
            yT_view = yT.ap().rearrange("(mo p) t -> p mo t", p=128)
            for mo in range(KO):
                ps_y = ps_a.tile([128, S], f32, tag="ps_a", name="ps_y")
                for fo in range(KO):
                    nc.tensor.matmul(
                        ps_y[:], w2t16[:, fo, mo * 128:(mo + 1) * 128],
                        T16[:, fo, :],
                        start=(fo == 0), stop=(fo == KO - 1))
                if mo < KO - 1:
                    # alternate output queues: DRAM-write drain runs on both
                    y_t = ypool.tile([128, S], f32, tag="y", name="y_t")
                    nc.scalar.activation(
                        y_t[:], ps_y[:],
                        mybir.ActivationFunctionType.Identity,
                        bias=b2_s[:, mo:mo + 1], scale=1.0)
                    eng = nc.sync if (mo & 1) == 0 else nc.scalar
                    eng.dma_start(yT_view[:, mo, :], y_t[:])
                else:
                    # last chunk: one full-width bias-add on Vector, then
                    # PARTITION-sliced halves (keeps 2KB DMA rows) drain in
                    # parallel on both queues
                    y_a = ypool.tile([128, S], f32, tag="y", name="y_a")
                    nc.vector.tensor_scalar_add(
                        out=y_a[:], in0=ps_y[:],
                        scalar1=b2_s[:, mo:mo + 1])
                    nc.sync.dma_start(yT_view[0:64, mo, :], y_a[0:64, :])
                    nc.scalar.dma_start(yT_view[64:128, mo, :], y_a[64:128, :])

    nc.compile()
    return nc


def _prep_inputs(x, w_qkv, b_qkv, w_out, b_out):
    x = np.asarray(x, dtype=np.float32)
    w_qkv = np.asarray(w_qkv, dtype=np.float32)
    b_qkv = np.asarray(b_qkv, dtype=np.float32)
    w_out = np.asarray(w_out, dtype=np.float32)
    b_out = np.asarray(b_out, dtype=np.float32)

    Wq = w_qkv[:, :L]
    Wk = w_qkv[:, L:2 * L]
    Wv = w_qkv[:, 2 * L:]
    bq = b_qkv[:L]
    bv = b_qkv[2 * L:]

    G = Wk @ Wq.T                    # [L, L]
    cvec = Wk @ bq                   # [L]
    W2 = Wv @ w_out                  # [L, L]
    b2 = bv @ w_out + b_out          # [L]

    x16 = x.astype(np.float16)
    xT16 = x16.T                     # [L, N]

    # gt: per-a slice rows (a*128+bp) hold [bo, la] with
    # gt[a*128+bp, bo*128+la] = G[a*128+la, bo*128+bp]
    G16 = G.astype(np.float16).reshape(KO, 128, KO, 128)   # [a, la, bo, bp]
    gt_host = np.ascontiguousarray(
        G16.transpose(0, 3, 2, 1).reshape(KO, 128, KO * 128))  # [a, bp, bo*la]

    # w2t: [p][fo][m]
    w2_host = np.ascontiguousarray(
        W2.astype(np.float16).reshape(KO, 128, L)
        .transpose(1, 0, 2).reshape(128, KO * L))

    cp16 = np.ascontiguousarray(cvec.reshape(KO, 128).T.astype(np.float16))
    shared = {
        "cp": cp16,
        "gta0": gt_host[0],
        "gtr": np.ascontiguousarray(
            gt_host[1:].transpose(1, 0, 2).reshape(128, 7 * KO * 128)),
        "w2t": w2_host,
        "b2": np.ascontiguousarray(b2.reshape(KO, 128).T.astype(np.float32)),
    }
    in_maps = []
    for i in range(R):
        m = dict(shared)
        # rotate tokens so core i's own tokens are keys/cols 0..511;
        # softmax + attn@v are key-order invariant, so this is exact.
        xr = np.roll(xT16, -i * S, axis=1)          # [L, N] rotated
        # xt0: [p][h][lo][256] (own tokens, query-half pieces)
        m["xt0"] = np.ascontiguousarray(
            xr[:, :S].reshape(KO, 128, 2, SH)
            .transpose(1, 2, 0, 3).reshape(128, 2 * KO * SH))
        # xtr: [p][b][lo][512] blocks 1..7
        m["xtr"] = np.ascontiguousarray(
            xr[:, S:].reshape(KO, 128, 7, 512)
            .transpose(1, 2, 0, 3).reshape(128, 7 * KO * 512))
        # xtok: [p][kc][l] rotated rows
        xrk = np.roll(x16, -i * S, axis=0)
        m["xtok"] = np.ascontiguousarray(
            xrk.reshape(NKC, 128, L).transpose(1, 0, 2).reshape(128, NKC * L))
        in_maps.append(m)
    return in_maps


def kernel(x, w_qkv, b_qkv, w_out, b_out, trace=False, **run_kwargs):
    global _cached
    if _cached is None:
        _cached = _build()
    nc = _cached
    in_maps = _prep_inputs(x, w_qkv, b_qkv, w_out, b_out)
    res = run_bass_kernel_spmd(nc, in_maps, core_ids=list(range(R)),
                               trace=trace, **run_kwargs)
    y = np.concatenate(
        [res.results[i]["yT"].T for i in range(R)], axis=0)
    kernel.last_results = res
    return np.ascontiguousarray(y, dtype=np.float32)


# revision 21
# speedup vs baseline: 1.0180x; 1.0180x over previous
"""Trainium2 Bass kernel for nn_MultiHeadAttention_64106681860559.

Fused single-score-matrix MHA: qkv = x@Wqkv+b; S = q k^T/8; attn = softmax(S);
out = (attn @ v) @ Wout + bout.   x:[4096,1024] fp32 -> y:[4096,1024] fp32.

Strategy: shard queries (dim 0) across 8 cores; ZERO collectives via weight
folding (associativity):
  scores^T = K Q^T = x (Wk Wq^T) x_own^T   with G = Wk Wq^T folded on host,
  so per core: P = G^T-chunks @ x_own^T  [1024, 512], then S^T = x @ P using
  the full (replicated) x — no K/V AllGather needed.  The key-side bias
  (x_j . Wk bq) folds into P's bias add; query-side constants cancel in
  softmax.  Attention output:
  y^T = (Wv Wo)^T (x^T E) * (1/d) + (bv Wo + bo)  with W2 = Wv Wo folded on
  host; T = x^T E is accumulated unnormalized and the per-query 1/d scale is
  folded into T's final fp16 conversion (T*rb is the normalized attention
  output, O(1), fp16-safe), so the y evacuation is a single bias-add.
Token rotation: each core's x/x^T host layouts are rolled by i*512 tokens so
its OWN tokens are always keys 0..511 — softmax and attn@v are key-order
invariant, so this is exact, and it removes the separate x_own load while
letting the first P matmuls start after just cp|gt_a0|xt-half0 (~0.77 MB).
Schedule: PE warmup spins during the fixed kernel-entry + first-DMA window
(keeps the HAM clock gate open), critical loads are split across BOTH HW DGE
queues (sync + scalar) in exact consumption order, all DMA sources are
host-relaid-out for >=2KB contiguous runs.
Per-core PE work: P (32768 cyc) + S^T (131072) + T (131072) + y^T (32768)
= 327680 cycles of fp16 matmul (~137 us at 2.4 GHz).
"""
import sys
import numpy as np

for _p in ("/opt/trn_rl_repo", "/root/.axon_site/_ro/trn_rl_repo"):
    if _p not in sys.path:
        sys.path.insert(0, _p)

import concourse.bass as bass  # noqa: E402
import concourse.tile as tile  # noqa: E402
from concourse import bacc, mybir  # noqa: E402
from concourse.bass_utils import run_bass_kernel_spmd  # noqa: E402

R = 8            # cores
N = 4096         # tokens
S = N // R       # 512 queries per shard
SH = S // 2      # 256-query half
L = 1024         # latent
KO = L // 128    # 8 latent chunks
NKC = N // 128   # 32 key chunks
NQT = 4          # key-chunk quarters (8 chunks each)
NB = N // 512    # 8 token blocks of 512
EXP_SHIFT = -16.0
SCALE = 0.125    # 1/sqrt(Dk)
WARMUP = 100

f16 = mybir.dt.float16
f32 = mybir.dt.float32

_cached = None


def _build():
    nc = bacc.Bacc("TRN2", target_bir_lowering=False, debug=False, num_devices=R)

    # all host views pre-laid-out partition-major for contiguous DMA
    cp = nc.dram_tensor("cp", [128, KO], f16, kind="ExternalInput")
    gta0 = nc.dram_tensor("gta0", [128, KO * 128], f16, kind="ExternalInput")
    gtr = nc.dram_tensor("gtr", [128, 7 * KO * 128], f16, kind="ExternalInput")
    # xt0: own-token block, piece-major: [p][h][lo][256]
    xt0 = nc.dram_tensor("xt0", [128, 2 * KO * SH], f16, kind="ExternalInput")
    # xtr: blocks 1..7, [p][b][lo][512]
    xtr = nc.dram_tensor("xtr", [128, 7 * KO * 512], f16, kind="ExternalInput")
    xtok = nc.dram_tensor("xtok", [128, NKC * L], f16, kind="ExternalInput")
    w2t = nc.dram_tensor("w2t", [128, KO * L], f16, kind="ExternalInput")
    b2 = nc.dram_tensor("b2", [128, KO], f32, kind="ExternalInput")
    yT = nc.dram_tensor("yT", [L, S], f32, kind="ExternalOutput")

    with tile.TileContext(nc) as tc:
        with tc.tile_pool(name="const", bufs=1) as const, \
             tc.tile_pool(name="xkpool", bufs=2) as xkpool, \
             tc.tile_pool(name="epool", bufs=16) as epool, \
             tc.tile_pool(name="ypool", bufs=3) as ypool, \
             tc.tile_pool(name="ps_a", bufs=2, space="PSUM") as ps_a, \
             tc.tile_pool(name="ps_s", bufs=3, space="PSUM") as ps_s_pool, \
             tc.tile_pool(name="ps_t", bufs=3, space="PSUM") as ps_t_pool:

            # ---- PE warmup: dummy matmuls during the fixed kernel-entry +
            #      first-DMA window keep the HAM activity monitor busy so P
            #      runs at full clock from its first instruction ----
            warm16 = const.tile([128, 64], f16, name="warm16")
            nc.vector.memset(warm16[:], 0.0)
            ps_w0 = ps_a.tile([128, 64], f32, tag="ps_a", name="ps_w0")
            ps_w1 = ps_a.tile([128, 64], f32, tag="ps_a", name="ps_w1")
            for i in range(WARMUP):
                ps_w = ps_w0 if (i & 1) == 0 else ps_w1
                nc.tensor.matmul(ps_w[:64, :], warm16[:, :64],
                                 warm16[:, :64], start=True, stop=True)

            # ---- sync-queue (q1) loads, strict need order.  Early DMA
            #      bandwidth is a shared ~220 GB/s pool no matter how many
            #      queues are active, so ALL head-critical bytes go on ONE
            #      queue in exact consumption order; the scalar queue's bulk
            #      is held back until S^T starts (see below) ----
            cp16v = const.tile([128, KO], f16, name="cp16v")
            nc.sync.dma_start(cp16v[:], cp.ap())
            gta0_t = const.tile([128, KO, 128], f16, name="gta0_t")
            gta0_view = gta0.ap().rearrange("p (bo la) -> p bo la", la=128)
            # own-token block: [p][h][lo][256]
            xt0_t = const.tile([128, 2, KO, SH], f16, name="xt0_t")
            xt0_view = xt0.ap().rearrange(
                "p (h lo t) -> p h lo t", h=2, t=SH)
            nc.sync.dma_start(gta0_t[:], gta0_view)
            nc.sync.dma_start(xt0_t[:, 0], xt0_view[:, 0])
            gt16 = const.tile([128, 7, KO, 128], f16, name="gt16")
            gtr_view = gtr.ap().rearrange(
                "p (a bo la) -> p a bo la", a=7, la=128)
            for a in range(7):          # a = 1..7, all on sync queue
                nc.sync.dma_start(gt16[:, a], gtr_view[:, a])
            nc.sync.dma_start(xt0_t[:, 1], xt0_view[:, 1])

            cp_s = const.tile([128, KO], f32, name="cp_s")
            nc.vector.tensor_copy(out=cp_s[:], in_=cp16v[:])

            ones_c16 = const.tile([128, 1], f16, name="ones_c16")
            nc.vector.memset(ones_c16[:], 1.0)
            ones_r16 = const.tile([1, 128], f16, name="ones_r16")
            nc.vector.memset(ones_r16[:], 1.0)
            expb = const.tile([128, 1], f32, name="expb")
            nc.vector.memset(expb[:], EXP_SHIFT)

            P16 = const.tile([128, KO, S], f16, name="P16")
            xt16 = const.tile([128, 7, KO, 512], f16, name="xt16")
            t_sum = const.tile([128, S], f32, name="t_sum")
            T_sb = const.tile([128, KO, S], f32, name="T_sb")
            T16 = const.tile([128, KO, S], f16, name="T16")
            rb32 = const.tile([128, S], f32, name="rb32")
            recip16 = const.tile([1, S], f16, name="recip16")

            xtr_view = xtr.ap().rearrange(
                "p (b lo t) -> p b lo t", b=7, t=512)

            def load_xt(b):  # blocks 1..7
                nc.sync.dma_start(xt16[:, b - 1], xtr_view[:, b - 1])

            for b in range(1, 8):
                load_xt(b)
            b2_s = const.tile([128, KO], f32, name="b2_s")
            nc.sync.dma_start(b2_s[:], b2.ap())

            # ---- scalar-queue (q10) bulk: triggers are EMITTED later (after
            #      the first S^T exp on the scalar engine) so they can't
            #      steal early DMA bandwidth from the critical path ----
            xtok_view = xtok.ap().rearrange("p (kc l) -> p kc l", l=L)
            xq_tiles = []

            def load_xq(qt, gate=None):
                xq = xkpool.tile([128, 8, L], f16, tag="xq", name=f"xq{qt}")
                if gate is not None:
                    # 1-elem marker copy: forces the Tile scheduler to hold
                    # this trigger until `gate` is written (exp of an early
                    # S^T chain), keeping the early DMA pool for the
                    # head-critical sync-queue loads
                    nc.vector.tensor_copy(out=xq[0:1, 0, 0:1],
                                          in_=gate[0:1, 0:1])
                nc.scalar.dma_start(xq[:], xtok_view[:, qt * 8:(qt + 1) * 8, :])
                xq_tiles.append(xq)

            w2t16 = const.tile([128, KO, L], f16, name="w2t16")

            # ---- phase P (query-half h): P[:,:,h] = G x_own^T + cvec ----
            # own tokens are xt block 0 (host-rotated), piece h = queries half
            def p_chunk(a, h):
                ps = ps_a.tile([128, SH], f32, tag="ps_a", name="ps_p")
                for bo in range(KO):
                    lhs = (gta0_t[:, bo, :] if a == 0
                           else gt16[:, a - 1, bo, :])
                    nc.tensor.matmul(
                        ps[:], lhs, xt0_t[:, h, bo, :],
                        start=(bo == 0), stop=(bo == KO - 1))
                nc.vector.tensor_scalar_add(
                    out=P16[:, a, h * SH:(h + 1) * SH], in0=ps[:],
                    scalar1=cp_s[:, a:a + 1])

            # xt slice for key-chunk kc: kc 0..3 live in xt0_t, rest in xt16
            def xt_slice(kc, lo):
                if kc < 2:
                    return xt0_t[:, 0, lo, kc * 128:(kc + 1) * 128]
                if kc < 4:
                    return xt0_t[:, 1, lo, (kc - 2) * 128:(kc - 1) * 128]
                b = kc // 4
                c = kc % 4
                return xt16[:, b - 1, lo, c * 128:(c + 1) * 128]

            # ---- S^T chunk (full or half width) + exp + t_sum ----
            tsum_started = [False, False]
            e_tiles = {}

            def st_chain(kc, h=None):
                c0, c1 = (0, S) if h is None else (h * SH, (h + 1) * SH)
                w = c1 - c0
                ps_s = ps_s_pool.tile([128, S], f32, tag="ps_s", name="ps_s")
                for lo in range(KO):
                    nc.tensor.matmul(
                        ps_s[:, :w], xt_slice(kc, lo),
                        P16[:, lo, c0:c1],
                        start=(lo == 0), stop=(lo == KO - 1))
                if kc in e_tiles:
                    e_t = e_tiles[kc]
                else:
                    e_t = epool.tile([128, S], f16, tag="e", name="e_t")
                    e_tiles[kc] = e_t
                nc.scalar.activation(
                    e_t[:, c0:c1], ps_s[:, :w],
                    mybir.ActivationFunctionType.Exp,
                    bias=expb[:], scale=SCALE)
                if h is None and all(tsum_started):
                    nc.vector.tensor_tensor(
                        t_sum[:], t_sum[:], e_t[:], mybir.AluOpType.add)
                else:
                    for hh in ([0, 1] if h is None else [h]):
                        hc0, hc1 = hh * SH, (hh + 1) * SH
                        if not tsum_started[hh]:
                            nc.vector.tensor_copy(
                                out=t_sum[:, hc0:hc1], in_=e_t[:, hc0:hc1])
                            tsum_started[hh] = True
                        else:
                            nc.vector.tensor_tensor(
                                t_sum[:, hc0:hc1], t_sum[:, hc0:hc1],
                                e_t[:, hc0:hc1], mybir.AluOpType.add)

            def t_pass(qt, xq, lo_range=None):
                last = (qt == NQT - 1)
                for lo in (lo_range if lo_range is not None else range(KO)):
                    ps_t = ps_t_pool.tile([128, S], f32, tag="ps_t", name="ps_t")
                    for j in range(8):
                        kc = qt * 8 + j
                        nc.tensor.matmul(
                            ps_t[:], xq[:, j, lo * 128:(lo + 1) * 128],
                            e_tiles[kc][:],
                            start=(j == 0), stop=(j == 7))
                    if qt == 0:
                        nc.vector.tensor_copy(out=T_sb[:, lo, :], in_=ps_t[:])
                    elif not last:
                        nc.vector.tensor_tensor(
                            T_sb[:, lo, :], T_sb[:, lo, :], ps_t[:],
                            mybir.AluOpType.add)
                    else:   # final quarter: add, then fold 1/d into the
                            # fp16 convert (T*rb = normalized attn output)
                        nc.vector.tensor_tensor(
                            T_sb[:, lo, :], T_sb[:, lo, :], ps_t[:],
                            mybir.AluOpType.add)
                        nc.vector.tensor_tensor(
                            T16[:, lo, :], T_sb[:, lo, :], rb32[:],
                            mybir.AluOpType.mult)

            # ---- pipeline: P h0 -> S^T(q0,h0) -> P h1 -> S^T(q0,h1) ->
            #      T(q0) -> S^T(q1) -> T(q1) -> S^T(q2) -> S^T(q3) ->
            #      T(q2) -> T(q3).  The last 16 exps get the whole T(q2/q3)
            #      window to drain before the denominator reads t_sum ----
            for a in range(KO):
                p_chunk(a, 0)
            for kc in range(5):
                st_chain(kc, h=0)
            # bulk loads, each gated behind an early exp via marker copies
            for qt in range(NQT):
                load_xq(qt, gate=e_tiles[qt])
            nc.vector.tensor_copy(out=w2t16[0:1, 0, 0:1],
                                  in_=e_tiles[4][0:1, 0:1])
            nc.scalar.dma_start(
                w2t16[:], w2t.ap().rearrange("p (fo m) -> p fo m", m=L))
            for kc in range(5, 8):
                st_chain(kc, h=0)
            for a in range(KO):
                p_chunk(a, 1)
            for kc in range(8):
                st_chain(kc, h=1)
            t_pass(0, xq_tiles[0])
            for kc in range(8, 16):
                st_chain(kc)
            t_pass(1, xq_tiles[1])
            for kc in range(16, 32):
                st_chain(kc)

            # ---- denominator chain, interleaved so the slow one-lane
            #      reciprocal (~3.3us, on idle ps_a banks to avoid PSUM
            #      port contention) hides under t_pass(2), and the
            #      broadcast matmul issues only after it completes ----
            t_pass(2, xq_tiles[2], lo_range=range(0, 1))
            # fp16 copy of t_sum so the denominator sum-matmul takes the
            # fast fp16 path (fp32 stationary forces the 4x-slower mode)
            t_sum16 = const.tile([128, S], f16, name="t_sum16")
            with nc.allow_low_precision(reason="softmax denominator sum in fp16; 2^-11 rel err far under tolerance"):
                nc.vector.tensor_copy(out=t_sum16[:], in_=t_sum[:])
            psum_d = ps_a.tile([1, S], f32, tag="ps_a", name="psum_d")
            nc.tensor.matmul(psum_d[:], ones_c16[:], t_sum16[:],
                             start=True, stop=True)
            with nc.allow_low_precision(reason="1/d broadcast in fp16; 2^-11 rel err on softmax denominators is far under tolerance"):
                nc.vector.reciprocal(out=recip16[:], in_=psum_d[:])
            t_pass(2, xq_tiles[2], lo_range=range(1, KO))
            ps_bc = ps_a.tile([128, S], f32, tag="ps_a", name="ps_bc")
            nc.tensor.matmul(ps_bc[:], ones_r16[:], recip16[:],
                             start=True, stop=True)
            nc.vector.tensor_copy(out=rb32[:], in_=ps_bc[:])
            t_pass(3, xq_tiles[3])

            # ---- output projection: yT = W2^T (T*rb) + b2; the 1/d scale
            #      was folded into T16, so evacuation is one bias-add ----
            yT_view = yT.ap().rearrange("(mo p) t -> p mo t", p=128)
            for mo in range(KO):
                ps_y = ps_a.tile([128, S], f32, tag="ps_a", name="ps_y")
                for fo in range(KO):
                    nc.tensor.matmul(
                        ps_y[:], w2t16[:, fo, mo * 128:(mo + 1) * 128],
                        T16[:, fo, :],
                        start=(fo == 0), stop=(fo == KO - 1))
                if mo < KO - 1:
                    # alternate output queues: DRAM-write drain runs on both
                    y_t = ypool.tile([128, S], f32, tag="y", name="y_t")
                    nc.scalar.activation(
                        y_t[:], ps_y[:],
                        mybir.ActivationFunctionType.Identity,
                        bias=b2_s[:, mo:mo + 1], scale=1.0)
                    eng = nc.sync if (mo & 1) == 0 else nc.scalar
                    eng.dma_start(yT_view[:, mo, :], y_t[:])
                else:
                    # last chunk: halves evac on DIFFERENT compute engines
                    # and drain in PARALLEL on both DMA queues (column split
                    # = more, smaller descriptors; the tail drain is
                    # descriptor-latency bound, so this beats row split)
                    y_a = ypool.tile([128, SH], f32, tag="y", name="y_a")
                    nc.vector.tensor_scalar_add(
                        out=y_a[:], in0=ps_y[:, 0:SH],
                        scalar1=b2_s[:, mo:mo + 1])
                    nc.sync.dma_start(yT_view[:, mo, 0:SH], y_a[:])
                    y_b = ypool.tile([128, SH], f32, tag="y", name="y_b")
                    nc.scalar.activation(
                        y_b[:], ps_y[:, SH:S],
                        mybir.ActivationFunctionType.Identity,
                        bias=b2_s[:, mo:mo + 1], scale=1.0)
                    nc.scalar.dma_start(yT_view[:, mo, SH:S], y_b[:])

    nc.compile()
    return nc


def _prep_inputs(x, w_qkv, b_qkv, w_out, b_out):
    x = np.asarray(x, dtype=np.float32)
    w_qkv = np.asarray(w_qkv, dtype=np.float32)
    b_qkv = np.asarray(b_qkv, dtype=np.float32)
    w_out = np.asarray(w_out, dtype=np.float32)
    b_out = np.asarray(b_out, dtype=np.float32)

    Wq = w_qkv[:, :L]
    Wk = w_qkv[:, L:2 * L]
    Wv = w_qkv[:, 2 * L:]
    bq = b_qkv[:L]
    bv = b_qkv[2 * L:]

    G = Wk @ Wq.T                    # [L, L]
    cvec = Wk @ bq                   # [L]
    W2 = Wv @ w_out                  # [L, L]
    b2 = bv @ w_out + b_out          # [L]

    x16 = x.astype(np.float16)
    xT16 = x16.T                     # [L, N]

    # gt: per-a slice rows (a*128+bp) hold [bo, la] with
    # gt[a*128+bp, bo*128+la] = G[a*128+la, bo*128+bp]
    G16 = G.astype(np.float16).reshape(KO, 128, KO, 128)   # [a, la, bo, bp]
    gt_host = np.ascontiguousarray(
        G16.transpose(0, 3, 2, 1).reshape(KO, 128, KO * 128))  # [a, bp, bo*la]

    # w2t: [p][fo][m]
    w2_host = np.ascontiguousarray(
        W2.astype(np.float16).reshape(KO, 128, L)
        .transpose(1, 0, 2).reshape(128, KO * L))

    cp16 = np.ascontiguousarray(cvec.reshape(KO, 128).T.astype(np.float16))
    shared = {
        "cp": cp16,
        "gta0": gt_host[0],
        "gtr": np.ascontiguousarray(
            gt_host[1:].transpose(1, 0, 2).reshape(128, 7 * KO * 128)),
        "w2t": w2_host,
        "b2": np.ascontiguousarray(b2.reshape(KO, 128).T.astype(np.float32)),
    }
    in_maps = []
    for i in range(R):
        m = dict(shared)
        # rotate tokens so core i's own tokens are keys/cols 0..511;
        # softmax + attn@v are key-order invariant, so this is exact.
        xr = np.roll(xT16, -i * S, axis=1)          # [L, N] rotated
        # xt0: [p][h][lo][256] (own tokens, query-half pieces)
        m["xt0"] = np.ascontiguousarray(
            xr[:, :S].reshape(KO, 128, 2, SH)
            .transpose(1, 2, 0, 3).reshape(128, 2 * KO * SH))
        # xtr: [p][b][lo][512] blocks 1..7
        m["xtr"] = np.ascontiguousarray(
            xr[:, S:].reshape(KO, 128, 7, 512)
            .transpose(1, 2, 0, 3).reshape(128, 7 * KO * 512))
        # xtok: [p][kc][l] rotated rows
        xrk = np.roll(x16, -i * S, axis=0)
        m["xtok"] = np.ascontiguousarray(
            xrk.reshape(NKC, 128, L).transpose(1, 0, 2).reshape(128, NKC * L))
        in_maps.append(m)
    return in_maps


def kernel(x, w_qkv, b_qkv, w_out, b_out, trace=False, **run_kwargs):
    global _cached
    if _cached is None:
        _cached = _build()
    nc = _cached
    in_maps = _prep_inputs(x, w_qkv, b_qkv, w_out, b_out)
    res = run_bass_kernel_spmd(nc, in_maps, core_ids=list(range(R)),
                               trace=trace, **run_kwargs)
    y = np.concatenate(
        [res.results[i]["yT"].T for i in range(R)], axis=0)
    kernel.last_results = res
    return np.ascontiguousarray(y, dtype=np.float32)
